# revision 45
# baseline (speedup 1.0000x reference)
"""Trainium2 Bass kernel for nn_Block_76519137345684 (Spikformer-style block:
spiking self-attention + spiking gated MLP with training-mode BatchNorm).

Strategy
- Data-parallel over batch B across 8 NeuronCores (16 batch each). BN batch
  statistics (per-channel sum / sum-of-squares) are AllReduced across cores.
- Activations live channel-on-partition: (C, rows) with rows
  r = ((t*16 + b)*64 + n); LIF timesteps are contiguous 1024-column slices.
- BN application is FUSED into the LIF recurrence on DVE in f16:
      yh_t = A*z_t + C   (per-partition A=0.5*a, C=0.5*c as tensor_scalar
                          AP operands - scalar APs don't break DVE fast modes)
      u_t  = 0.5*u_{t-1}*[u_{t-1} < thr] + yh_t,  s_t = [u_t >= thr]
- Attention uses associativity: y = q @ (k^T v) * scale; per-head block
  structure enforced with a 0.125-scaled block-diagonal mask.
- Depthwise 3x3 conv on the TENSOR engine: spikes written into a zero-padded
  plane (10 rows x 10 cols per frame); 9 taps = 9 PSUM-accumulated matmuls
  with diagonal per-channel weights against shifted plane views. PSUM chunks
  are 4 frames (400 cols) and evacuate DIRECTLY to the contiguous z layout
  via strided ACT copy with fused stats accumulation.
- Matmul dtypes: f32r for continuous-input layers (q,k,v,fc1), fp16 for
  binary-input layers (p, fc2, conv) and attention.
- xmid (residual) stays in SBUF through fc1; spilled to DRAM only for the
  final residual read.
"""
import sys
sys.path.insert(0, '/opt/trn_rl_repo')
import numpy as np

import concourse.bass as bass
import concourse.mybir as mybir
import concourse.tile as tile
from concourse.tile import add_dep_helper

T, B, N, C = 4, 128, 64, 384
HID, CH, HEADS, HD = 1536, 768, 12, 32
NCORES = 8
BS = B // NCORES
R = T * BS * N              # 4096 rows per core
TC = BS * N                 # 1024 cols per timestep
COUNT = T * B * N           # 32768 rows globally (BN stat count)
EPS = 1e-5
PADW = 10
PADP = PADW * 10            # 100 per frame plane
NFR = T * BS                # 64 frames
GUARD = 16
PLANE = NFR * PADP          # 6400
PADL = GUARD + PLANE + GUARD
FR_CH = 4                   # frames per conv psum chunk
CCH = NFR // FR_CH          # 16 chunks per tile
PCOLS = FR_CH * PADP        # 400 plane cols per chunk
ZCOLS = FR_CH * 64          # 256 z cols per chunk

F32 = mybir.dt.float32
F32R = mybir.dt.float32r
F16 = mybir.dt.float16
ALU = mybir.AluOpType
ACTF = mybir.ActivationFunctionType

_ctr = [0]


def _fix_multiwaits(nc):
    """walrus here accepts max 1 sync-wait per instruction: split extras
    onto same-engine NOPs."""
    for f in nc.m.functions:
        for bb in f.blocks:
            new_insts = []
            for inst in bb.instructions:
                si = inst.sync_info
                ow = list(si.on_wait) if (si and si.on_wait) else []
                if len(ow) > 1:
                    for w in ow[:-1]:
                        _ctr[0] += 1
                        new_insts.append(mybir.InstNoOp(
                            name=f"I-waitnop-{_ctr[0]}", engine=inst.engine,
                            sync_info=mybir.SyncInfo(on_wait=[w], on_update=[]),
                            bass_nofuse=True))
                    si.on_wait = [ow[-1]]
                new_insts.append(inst)
            bb.instructions[:] = new_insts


def build_kernel(debug_taps=False, timing=False, stop_after=None):
    nc = bass.Bass("TRN2", target_bir_lowering=False, debug=False,
                   num_devices=NCORES)

    xT_in = nc.declare_dram_parameter("xT", [C, R], F32R, isOutput=False)
    w_in = {}
    for name, ci, co, dt in [("q", C, C, F32R), ("k", C, C, F32R),
                             ("v", C, C, F32R), ("p", C, C, F16),
                             ("fc1", C, HID, F32R), ("fc2", CH, C, F16)]:
        w_in[name] = nc.declare_dram_parameter(f"w_{name}", [ci, co], dt,
                                               isOutput=False)
    pv_in = {}
    for name, co in [("q", C), ("k", C), ("v", C), ("p", C),
                     ("fc1", HID), ("dw", CH), ("fc2", C)]:
        pv_in[name] = nc.declare_dram_parameter(f"pv_{name}", [co, 2], F32,
                                                isOutput=False)
    ident_in = nc.declare_dram_parameter("ident", [128, 128], F16, isOutput=False)
    mask_in = nc.declare_dram_parameter("mask", [128, 512], F16, isOutput=False)
    # 54 diagonal [128,128] f16 weight matrices: (tile i, tap k) at row
    # (i*9+k)*128
    convd_in = nc.declare_dram_parameter("convd", [54 * 128, 128], F16,
                                         isOutput=False)
    if timing:
        out_d = nc.dram_tensor("out", [C, R], F32)
        tok_d = nc.declare_dram_parameter("tok", [128, 1], F32, isOutput=True)
    else:
        out_d = nc.declare_dram_parameter("out", [C, R], F32, isOutput=True)
        tok_d = None

    dbg = {}
    if debug_taps:
        for nm, npt, dt in [("z_q", 3, F16), ("s_q", 3, F16), ("s_k", 3, F16),
                            ("s_v", 3, F16), ("z_y", 3, F16), ("s_y", 3, F16),
                            ("z_p", 3, F16), ("xmid", 3, F32),
                            ("z_fc1", 12, F16), ("z_conv", 6, F16),
                            ("s_conv", 6, F16), ("gated", 6, F16),
                            ("z_fc2", 3, F16)]:
            dbg[nm] = nc.declare_dram_parameter(f"dbg_{nm}", [npt * 128, R],
                                                dt, isOutput=True)

    cc = {}
    for name, co in [("qk", 2 * C), ("v", C), ("p", C),
                     ("fc1", HID), ("dw", CH), ("fc2", C)]:
        cci = nc.dram_tensor(f"cci_{name}", [co, 2], F32)
        cco = nc.dram_tensor(f"cco_{name}", [co, 2], F32, addr_space="Shared")
        cc[name] = (cci, cco)

    xmid_sp = nc.dram_tensor("xmid_spill", [C, R], F32)
    zx2_sp = nc.dram_tensor("zx2_spill", [CH, R], F16)

    with tile.TileContext(nc, pool_alloc_mode="queue") as tc:
        _body(nc, tc, xT_in, w_in, pv_in, ident_in, mask_in, convd_in, out_d,
              tok_d, cc, xmid_sp, zx2_sp, dbg, stop_after)
    _fix_multiwaits(nc)
    return nc


def _body(nc, tc, xT_in, w_in, pv_in, ident_in, mask_in, convd_in, out_d,
          tok_d, cc, xmid_sp, zx2_sp, dbg, stop_after=None):
    from contextlib import ExitStack

    # spike DRAM buffers (cross-phase hand-off)
    s_d = {name: nc.dram_tensor(f"s{name}_d", [C, R], F16)
           for name in ("q", "k", "v", "y")}

    # ---------- long-lived small pools ----------
    ctxL = ExitStack()
    const_p = ctxL.enter_context(tc.tile_pool(name="const", bufs=1))
    stat_p = ctxL.enter_context(tc.tile_pool(name="stats", bufs=1))
    scr_p = ctxL.enter_context(tc.tile_pool(name="scr", bufs=2))
    lif_p = ctxL.enter_context(tc.tile_pool(name="lifp", bufs=1))
    ps_mm = ctxL.enter_context(tc.tile_pool(name="psmm", bufs=4, space="PSUM"))
    ps_at = ctxL.enter_context(tc.tile_pool(name="psat", bufs=1, space="PSUM"))

    ident = const_p.tile([128, 128], F16, tag="ident", name="ident")
    nc.sync.dma_start(ident[:], ident_in[:])
    mask = const_p.tile([128, 512], F16, tag="mask", name="mask")
    nc.sync.dma_start(mask[:], mask_in[:])
    pvec = {}
    for name, npt in [("q", 3), ("k", 3), ("v", 3), ("p", 3),
                      ("fc1", 12), ("dw", 6), ("fc2", 3)]:
        pv = const_p.tile([128, 2 * npt], F32, tag=f"pv_{name}",
                          name=f"pv_{name}")
        for i in range(npt):
            nc.sync.dma_start(pv[:, 2 * i:2 * i + 2],
                              pv_in[name][128 * i:128 * (i + 1), :])
        pvec[name] = pv

    # stats: sump, sqp [128, ngrp*npt]; stfin [128, 2*npt] blocked (S | Q);
    # AC [128, 2*npt] blocked (A | C)
    NGRP = {"q": 8, "k": 8, "v": 8, "p": 8, "fc1": 8, "fc2": 8, "dw": 16}
    NPT = {"q": 3, "k": 3, "v": 3, "p": 3, "fc1": 12, "dw": 6, "fc2": 3}
    STT = {}
    for name, npt in [("q", 3), ("k", 3), ("v", 3), ("p", 3),
                      ("fc1", 12), ("dw", 6), ("fc2", 3)]:
        g = NGRP[name]
        STT[name] = (
            stat_p.tile([128, g * npt], F32, tag=f"sum_{name}", name=f"sum_{name}"),
            stat_p.tile([128, g * npt], F32, tag=f"sq_{name}", name=f"sq_{name}"),
            stat_p.tile([128, 2 * npt], F32, tag=f"st_{name}", name=f"st_{name}"),
            stat_p.tile([128, 2 * npt], F32, tag=f"ac_{name}", name=f"ac_{name}"),
        )

    # ---------- helpers ----------
    def emit_linear(name, wt, rhs, co_lo, co_hi, n_ci, z_alloc, z_done):
        """z tiles are f16. PSUM evac split between ACT and DVE per chunk so
        neither engine starves the PE; sq-sums on the other engine."""
        sump, sqp = STT[name][0], STT[name][1]
        for co in range(co_lo, co_hi):
            z = z_alloc(co)
            for cg in range(2):
                pss = [ps_mm.tile([128, 512], F32, tag="ps", name=f"ps{name}{co}{cg}{j}")
                       for j in range(4)]
                for ci in range(n_ci):
                    for ch in range(4):
                        nc.tensor.matmul(
                            pss[ch][:],
                            lhsT=wt[ci][:, co * 128:(co + 1) * 128],
                            rhs=rhs[ci][:, (cg * 4 + ch) * 512:(cg * 4 + ch + 1) * 512],
                            start=(ci == 0), stop=(ci == n_ci - 1))
                for ch in range(4):
                    g = cg * 4 + ch
                    sl = slice(g * 512, (g + 1) * 512)
                    sq = scr_p.tile([128, 512], F16, tag="sqscr", name="sqscr")
                    if g % 2 == 0:
                        nc.scalar.activation(z[:, sl], pss[ch][:], ACTF.Copy,
                                             accum_out=sump[:, co * 8 + g:co * 8 + g + 1])
                        nc.vector.scalar_tensor_tensor(
                            sq[:], z[:, sl], 1.0, z[:, sl], ALU.mult, ALU.mult,
                            accum_out=sqp[:, co * 8 + g:co * 8 + g + 1])
                    else:
                        nc.vector.tensor_scalar(
                            z[:, sl], pss[ch][:], 1.0, 0.0, ALU.mult, ALU.add,
                            accum_out=sump[:, co * 8 + g:co * 8 + g + 1])
                        nc.scalar.activation(
                            sq[:], pss[ch][:], ACTF.Square,
                            accum_out=sqp[:, co * 8 + g:co * 8 + g + 1])
            z_done(co, z)

    def _ar_reduce_in(name, lo, hi, cci, row0, dmas):
        """tensor_reduce per-ptile stats into stfin (blocked S|Q) and DMA
        them into `cci` starting at block row0. Stats DMAs ride the idle
        GpSimd queue so they don't wait behind bulk spills on Sync."""
        ngrp = NGRP[name]
        npt = NPT[name]
        sump, sqp, stfin, _ = STT[name]
        for co in range(lo, hi):
            nc.vector.tensor_reduce(stfin[:, co:co + 1],
                                    sump[:, co * ngrp:(co + 1) * ngrp],
                                    axis=mybir.AxisListType.X, op=ALU.add)
            nc.vector.tensor_reduce(stfin[:, npt + co:npt + co + 1],
                                    sqp[:, co * ngrp:(co + 1) * ngrp],
                                    axis=mybir.AxisListType.X, op=ALU.add)
            r = 128 * (row0 + co - lo)
            dmas.append(nc.gpsimd.dma_start(cci[r:r + 128, 0:1],
                                            stfin[:, co:co + 1]))
            dmas.append(nc.gpsimd.dma_start(cci[r:r + 128, 1:2],
                                            stfin[:, npt + co:npt + co + 1]))

    def _ar_read_back(name, lo, hi, cco, row0):
        npt = NPT[name]
        stfin = STT[name][2]
        outs = []
        for co in range(lo, hi):
            r = 128 * (row0 + co - lo)
            outs.append(nc.gpsimd.dma_start(stfin[:, co:co + 1],
                                            cco[r:r + 128, 0:1]))
            outs.append(nc.gpsimd.dma_start(stfin[:, npt + co:npt + co + 1],
                                            cco[r:r + 128, 1:2]))
        return outs

    def emit_ar(name, lo, hi, key=None):
        """AllReduce stats for ptiles [lo, hi) of `name`."""
        cci, cco = cc[key or name]
        dmas = []
        _ar_reduce_in(name, lo, hi, cci, lo, dmas)
        ar = nc.gpsimd.collective_compute(
            "AllReduce", ALU.add, replica_groups=[list(range(NCORES))],
            ins=[cci[128 * lo:128 * hi, :]], outs=[cco[128 * lo:128 * hi, :]])
        for d in dmas:
            add_dep_helper(ar.ins, d.ins, reason="ar waits dma_in")
        for d in _ar_read_back(name, lo, hi, cco, lo):
            add_dep_helper(d.ins, ar.ins, reason="readback waits ar")

    def emit_ar_names(names, key):
        """ONE AllReduce covering several names' full stats — avoids the
        per-collective wakeup/stagger on the CC core."""
        cci, cco = cc[key]
        dmas = []
        row0 = 0
        for name in names:
            _ar_reduce_in(name, 0, NPT[name], cci, row0, dmas)
            row0 += NPT[name]
        ar = nc.gpsimd.collective_compute(
            "AllReduce", ALU.add, replica_groups=[list(range(NCORES))],
            ins=[cci[:]], outs=[cco[:]])
        for d in dmas:
            add_dep_helper(ar.ins, d.ins, reason="ar waits dma_in")
        row0 = 0
        for name in names:
            for d in _ar_read_back(name, 0, NPT[name], cco, row0):
                add_dep_helper(d.ins, ar.ins, reason="readback waits ar")
            row0 += NPT[name]

    def emit_params(name, lo, hi):
        """Batched: compute A = 0.5*a into AC[:, lo:hi] and C = 0.5*c into
        AC[:, npt+lo:npt+hi] with [128, n]-wide ops."""
        npt = NPT[name]
        _, _, stfin, AC = STT[name]
        pv = pvec[name]
        n = hi - lo
        S = stfin[:, lo:hi]
        Q = stfin[:, npt + lo:npt + hi]
        pvv = pv[:, 2 * lo:2 * hi].rearrange("p (n t) -> p t n", t=2)
        g_ = pvv[:, 0, :]
        be_ = pvv[:, 1, :]
        w = scr_p.tile([128, 6 * n], F32, tag="pscr", name="pscr")
        mean, qm, var, sd, inv, a_ = (w[:, j * n:(j + 1) * n] for j in range(6))
        nc.vector.tensor_scalar(mean, S, 1.0 / COUNT, None, ALU.mult)
        nc.vector.tensor_scalar(qm, Q, 1.0 / COUNT, None, ALU.mult)
        nc.vector.tensor_tensor(var, mean, mean, ALU.mult)
        nc.vector.tensor_tensor(var, qm, var, ALU.subtract)
        nc.vector.tensor_scalar(var, var, EPS, None, ALU.add)
        nc.scalar.sqrt(sd, var)
        nc.vector.reciprocal(inv, sd)
        nc.vector.tensor_tensor(a_, g_, inv, ALU.mult)
        nc.vector.tensor_scalar(AC[:, lo:hi], a_, 0.5, None, ALU.mult)
        # C = 0.5*(be - mean*a)
        nc.vector.tensor_tensor(qm, mean, a_, ALU.mult)
        nc.vector.tensor_tensor(qm, be_, qm, ALU.subtract)
        nc.vector.tensor_scalar(AC[:, npt + lo:npt + hi], qm, 0.5, None,
                                ALU.mult)

    def emit_lif_multi(items, thr=1.0):
        """Fused norm+LIF over a GROUP of z tiles, ops interleaved across
        tiles so dependent DVE ops don't stall the pipeline back-to-back.
        items: list of (z, writer, name, pt). The BN affine yh_t = A*z_t + C
        runs on ACT; the recurrence on DVE.
        u_t = 0.5*u_{t-1}*[u_{t-1} < thr] + yh_t ; writer(t, u_ap) emits
        spikes."""
        n = len(items)
        us, scs, hs, y2s = [], [], [], []
        for j in range(n):
            us.append(lif_p.tile([128, TC], F16, tag=f"lifu{j}",
                                 name=f"lifu{j}", bufs=1))
            scs.append(lif_p.tile([128, TC], F16, tag=f"lifsc{j}",
                                  name=f"lifsc{j}", bufs=1))
            hs.append(lif_p.tile([128, TC], F16, tag=f"lifh{j}",
                                 name=f"lifh{j}", bufs=1))
            y2s.append(lif_p.tile([128, TC], F16, tag=f"lify{j}",
                                  name=f"lify{j}", bufs=1))

        def yh_chunk(j, dst, t):
            z, _, name, pt = items[j]
            if name is None:
                return z[:, t * TC:(t + 1) * TC]
            AC = STT[name][3]
            npt = NPT[name]
            nc.scalar.activation(dst[:], z[:, t * TC:(t + 1) * TC],
                                 ACTF.Identity, scale=AC[:, pt:pt + 1],
                                 bias=AC[:, npt + pt:npt + pt + 1])
            return dst[:]

        ucur = [yh_chunk(j, us[j], 0) for j in range(n)]
        for t in range(T):
            for j in range(n):
                items[j][1](t, ucur[j])
            if t < T - 1:
                for j in range(n):
                    nc.vector.tensor_scalar(scs[j][:], ucur[j], thr, 0.5,
                                            ALU.is_lt, ALU.mult)
                yn = [yh_chunk(j, y2s[j], t + 1) for j in range(n)]
                for j in range(n):
                    nc.vector.tensor_tensor(hs[j][:], ucur[j], scs[j][:],
                                            ALU.mult)
                for j in range(n):
                    nc.vector.tensor_tensor(us[j][:], hs[j][:], yn[j],
                                            ALU.add)
                    ucur[j] = us[j][:]

    def emit_lif(z, writer, name=None, pt=0, thr=1.0):
        emit_lif_multi([(z, writer, name, pt)], thr=thr)

    def spike_writer(st, thr=1.0, eng=None):
        e = eng if eng is not None else nc.vector
        def w(t, ucur, st=st, thr=thr, e=e):
            e.tensor_scalar(st[:, t * TC:(t + 1) * TC], ucur, thr, None,
                            ALU.is_ge)
        return w

    def dump_rows(nm, row0, t_):
        if nm in dbg:
            nc.sync.dma_start(dbg[nm][row0:row0 + 128, :], t_[:])

    # ============ PHASE 1: q,k,v matmul + AR + LIF -> spikes to DRAM ======
    ctxA = ExitStack()
    pA = ctxA.enter_context(tc.tile_pool(name="pA", bufs=1))
    wts = {}
    for name in ("q", "k", "v"):
        wt = []
        for i in range(3):
            w = pA.tile([128, C], F32R, tag=f"w_{name}{i}", name=f"w_{name}{i}")
            nc.sync.dma_start(w[:], w_in[name][128 * i:128 * (i + 1), :])
            wt.append(w)
        wts[name] = wt
    xT = []
    for i in range(3):
        x = pA.tile([128, R], F32R, tag=f"xT{i}", name=f"xT{i}")
        for c4 in range(4):
            nc.sync.dma_start(x[:, 1024 * c4:1024 * (c4 + 1)],
                              xT_in[128 * i:128 * (i + 1),
                                    1024 * c4:1024 * (c4 + 1)])
        xT.append(x)

    zs = {}
    for name in ("q", "k", "v"):
        zt = []

        def zalloc(co, name=name, zt=zt):
            z = pA.tile([128, R], F16, tag=f"z{name}{co}", name=f"z{name}{co}",
                        bufs=1)
            zt.append(z)
            return z

        emit_linear(name, wts[name], xT, 0, 3, 3, zalloc, lambda co, z: None)
        zs[name] = zt
        if name == "k":
            # one AR for q+k stats, hidden under v's matmuls
            emit_ar_names(("q", "k"), "qk")
            emit_params("q", 0, 3)
            emit_params("k", 0, 3)
    emit_ar("v", 0, 3)
    emit_params("v", 0, 3)
    for pt in range(3):
        dump_rows("z_q", 128 * pt, zs["q"][pt])

    # LIF order k, v first so attention transposes can start while q runs;
    # each name's 3 ptiles run interleaved to keep the DVE pipe full
    for name in ("k", "v", "q"):
        sts = [pA.tile([128, R], F16, tag=f"spt{pt}", name=f"s{name}{pt}",
                       bufs=1) for pt in range(3)]
        emit_lif_multi([(zs[name][pt], spike_writer(sts[pt]), name, pt)
                        for pt in range(3)])
        for pt in range(3):
            nc.sync.dma_start(s_d[name][128 * pt:128 * (pt + 1), :],
                              sts[pt][:])
            dump_rows(f"s_{name}", 128 * pt, sts[pt])
    ctxA.close()
    if stop_after == 'qkv':
        ctxL.close(); return

    # ============ PHASE 2: transposes + attention + y-LIF ============
    ctxB = ExitStack()
    pB = ctxB.enter_context(tc.tile_pool(name="pB", bufs=1))
    # rm layout: per (pt, tb) a (128, 128) block at col (pt*64+tb)*128;
    # rows 0..63 = transposed spikes (n-major), rows 64..127 stay ZERO so
    # mm1 can contract over the full K=128 (K=64 matmuls hang on this HW).
    rm = {}
    for name in ("k", "v"):
        rmt = pB.tile([128, 6 * R], F16, tag=f"rm_{name}", name=f"rm_{name}")
        nc.gpsimd.memset(rmt[64:128, :], 0.0)
        for pt in range(3):
            srt = pB.tile([128, R], F16, tag=f"skvr{pt % 2}",
                          name=f"r{name}{pt}")
            nc.sync.dma_start(srt[:], s_d[name][128 * pt:128 * (pt + 1), :])
            for grp in range(8):
                pst = ps_at.tile([128, 1024], F16, tag="pstr", name="pstr")
                for j in range(8):
                    tb = grp * 8 + j
                    nc.tensor.transpose(pst[0:64, 128 * j:128 * (j + 1)],
                                        srt[:, 64 * tb:64 * (tb + 1)],
                                        ident[:])
                nc.scalar.copy(
                    rmt[0:64, (pt * 64 + grp * 8) * 128:(pt * 64 + grp * 8 + 8) * 128],
                    pst[0:64, :])
        rm[name] = rmt

    sy = []
    for pt in range(3):
        sqr = pB.tile([128, R], F16, tag=f"sqr{pt % 2}", name=f"sqr{pt}")
        nc.sync.dma_start(sqr[:], s_d["q"][128 * pt:128 * (pt + 1), :])
        zy = pB.tile([128, R], F16, tag=f"zy{pt % 2}", name=f"zy{pt}")
        for g4 in range(16):
            mm1ps = ps_at.tile([128, 512], F32, tag="mm1", name="mm1")
            for j in range(4):
                tb = g4 * 4 + j
                base = (pt * 64 + tb) * 128
                nc.tensor.matmul(mm1ps[:, 128 * j:128 * (j + 1)],
                                 lhsT=rm["k"][:, base:base + 128],
                                 rhs=rm["v"][:, base:base + 128],
                                 start=True, stop=True)
            m4 = scr_p.tile([128, 512], F16, tag="m4", name="m4")
            nc.vector.tensor_tensor(m4[:], mm1ps[:], mask[:], ALU.mult)
            yps = ps_at.tile([128, 256], F32, tag="yps", name="yps", bufs=2)
            for j in range(4):
                tb = g4 * 4 + j
                nc.tensor.matmul(yps[:, 64 * j:64 * (j + 1)],
                                 lhsT=m4[:, 128 * j:128 * (j + 1)],
                                 rhs=sqr[:, 64 * tb:64 * (tb + 1)],
                                 start=True, stop=True)
            # evacuate with 0.5 scale: zy holds Y = 0.5 * z_y
            nc.scalar.activation(zy[:, 256 * g4:256 * (g4 + 1)], yps[:],
                                 ACTF.Copy, scale=0.5)
        dump_rows("z_y", 128 * pt, zy)
        syt = pB.tile([128, R], F16, tag=f"sy{pt % 2}", name=f"sy{pt}")
        emit_lif(zy, spike_writer(syt, thr=0.5), thr=0.5)
        nc.sync.dma_start(s_d["y"][128 * pt:128 * (pt + 1), :], syt[:])
        dump_rows("s_y", 128 * pt, syt)
    ctxB.close()
    if stop_after == 'attn':
        ctxL.close(); return

    # ============ PHASE 3: p-linear + xmid (xr stays in SBUF for fc1) =====
    ctxZ = ExitStack()
    pZ = ctxZ.enter_context(tc.tile_pool(name="pZ", bufs=1))
    ctxC2 = ExitStack()
    pC2 = ctxC2.enter_context(tc.tile_pool(name="pC2", bufs=1))
    ctxC1 = ExitStack()
    pC1 = ctxC1.enter_context(tc.tile_pool(name="pC1", bufs=1))
    syr = []
    for i in range(3):
        s = pC1.tile([128, R], F16, tag=f"syr{i}", name=f"syr{i}")
        nc.sync.dma_start(s[:], s_d["y"][128 * i:128 * (i + 1), :])
        syr.append(s)
    wt_p = []
    for i in range(3):
        w = pC1.tile([128, C], F16, tag=f"w_p{i}", name=f"w_p{i}")
        nc.sync.dma_start(w[:], w_in["p"][128 * i:128 * (i + 1), :])
        wt_p.append(w)
    zp = []

    def zalloc_p(co):
        z = pC1.tile([128, R], F16, tag=f"zp{co}", name=f"zp{co}")
        zp.append(z)
        return z

    emit_linear("p", wt_p, syr, 0, 3, 3, zalloc_p, lambda co, z: None)
    emit_ar("p", 0, 3)
    emit_params("p", 0, 3)
    spts = [pC1.tile([128, R], F16, tag=f"sptr{pt}", name=f"sp{pt}", bufs=1)
            for pt in range(3)]
    # xr loads issued before the LIF so they fully overlap it
    # (xr written as f32r so the fc1 f32r matmul can consume it directly)
    xr_t = []
    for pt in range(3):
        xr = pC2.tile([128, R], F32R, tag=f"xm{pt}", name=f"xm{pt}")
        nc.sync.dma_start(xr[:], xT_in[128 * pt:128 * (pt + 1), :])
        xr_t.append(xr)
    for pt in range(3):
        dump_rows("z_p", 128 * pt, zp[pt])
    emit_lif_multi([(zp[pt], spike_writer(spts[pt]), "p", pt)
                    for pt in range(3)])
    for pt in range(3):
        # xr = x + p_spikes ; kept in SBUF for fc1, spilled for final residual
        xr = xr_t[pt]
        nc.vector.tensor_tensor(xr[:], xr[:].bitcast(F32), spts[pt][:],
                                ALU.add)
        nc.sync.dma_start(xmid_sp[128 * pt:128 * (pt + 1), :],
                          xr[:].bitcast(F32))
        dump_rows("xmid", 128 * pt, xr[:].bitcast(F32))
    ctxC1.close()
    if stop_after == 'p':
        ctxC2.close(); ctxZ.close(); ctxL.close(); return

    # ============ PHASE 4: fc1 (z tiles stay in SBUF) ============
    ctxD = ExitStack()
    pD = ctxD.enter_context(tc.tile_pool(name="pD", bufs=1))
    wt_fc1 = []
    for i in range(3):
        w = pD.tile([128, HID], F32R, tag=f"wfc1_{i}", name=f"wfc1_{i}")
        nc.sync.dma_start(w[:], w_in["fc1"][128 * i:128 * (i + 1), :])
        wt_fc1.append(w)
    xmid_v = [x[:] for x in xr_t]

    zf1 = {}

    def zalloc_f(co):
        if co < 6:
            z = pZ.tile([128, R], F16, tag=f"zf1_{co}", name=f"zf1_{co}")
            zf1[co] = z
        else:
            z = pD.tile([128, R], F16, tag=f"zx2_{co % 2}", name=f"zf1_{co}",
                        bufs=2)
        return z

    def zdone_f(co, z):
        if co >= 6:
            nc.sync.dma_start(zx2_sp[128 * (co - 6):128 * (co - 5), :], z[:])
        if "z_fc1" in dbg:
            nc.sync.dma_start(dbg["z_fc1"][128 * co:128 * (co + 1), :], z[:])

    # split the fc1 stats AR: x1-half (tiles 0-5) reduces while the x2-half
    # matmuls run, so x1-LIF + conv start much earlier
    emit_linear("fc1", wt_fc1, xmid_v, 0, 6, 3, zalloc_f, zdone_f)
    emit_ar("fc1", 0, 6)
    emit_params("fc1", 0, 6)
    emit_linear("fc1", wt_fc1, xmid_v, 6, 12, 3, zalloc_f, zdone_f)
    emit_ar("fc1", 6, 12)
    emit_params("fc1", 6, 12)
    ctxD.close()
    ctxC2.close()
    if stop_after == 'fc1':
        ctxZ.close(); ctxL.close(); return

    # ============ PHASE 5a: x1-LIF -> spike planes -> PE conv ============
    ctxE = ExitStack()
    pE = ctxE.enter_context(tc.tile_pool(name="pE", bufs=1))
    convd = pE.tile([128, 54 * 128], F16, tag="convd", name="convd")
    for i in range(54):
        nc.sync.dma_start(convd[:, 128 * i:128 * (i + 1)],
                          convd_in[128 * i:128 * (i + 1), :])
    # tap shift offsets in plane space, kh-major to match host convd order
    SHIFTS = [dh * PADW + dw for dh in (-1, 0, 1) for dw in (-1, 0, 1)]

    z_conv = []
    sx2_t = [None] * 6
    gated = [None] * 6
    sump_c, sqp_c, _, _ = STT["dw"]

    def conv_lif_gate3(lo):
        scvs = [pE.tile([128, R], F16, tag=f"scv{j % 3}", name=f"scv{j}",
                        bufs=1) for j in range(lo, lo + 3)]
        emit_lif_multi([(z_conv[j], spike_writer(scvs[j - lo]), "dw", j)
                        for j in range(lo, lo + 3)])
        for j in range(lo, lo + 3):
            dump_rows("s_conv", 128 * j, scvs[j - lo])
            g = pZ.tile([128, R], F16, tag=f"zf1_{j}", name=f"gated{j}")
            nc.vector.tensor_tensor(g[:], scvs[j - lo][:], sx2_t[j][:],
                                    ALU.mult)
            gated[j] = g
            dump_rows("gated", 128 * j, g)

    def x2_lif(i):
        zx2 = pE.tile([128, R], F16, tag="zx2r", name=f"zx2r{i}", bufs=2)
        nc.sync.dma_start(zx2[:], zx2_sp[128 * i:128 * (i + 1), :])
        sx2 = pE.tile([128, R], F16, tag=f"gt{i % 5}", name=f"sx2_{i}")
        emit_lif(zx2, spike_writer(sx2), name="fc1", pt=6 + i)
        sx2_t[i] = sx2

    for i in range(6):
        xa = pE.tile([128, PADL], F16, tag=f"cxa{i % 2}", name=f"cxa{i}")
        if i < 2:
            nc.gpsimd.memset(xa[:], 0.0)

        def x1_writer(t, ucur, xa=xa):
            # one strided is_ge into the padded plane per timestep
            xa4 = xa[:, GUARD + t * BS * PADP:GUARD + (t + 1) * BS * PADP] \
                .rearrange("p (f r w) -> p f r w", r=10, w=PADW)[:, :, 1:9, 1:9]
            u4 = ucur.rearrange("p (f h w) -> p f h w", h=8, w=8)
            nc.vector.tensor_scalar(xa4, u4, 1.0, None, ALU.is_ge)

        emit_lif(zf1[i], x1_writer, name="fc1", pt=i)

        # 9-tap depthwise conv via diagonal-weight matmuls; psum chunks of
        # FR_CH frames evacuate directly to contiguous z layout with stats
        zc = pZ.tile([128, R], F16, tag=f"zf1_{i}", name=f"zconv{i}")
        for c in range(CCH):
            cp = ps_mm.tile([128, 512], F32, tag="ps", name=f"cps{i}{c}")
            for k in range(9):
                base = GUARD + c * PCOLS + SHIFTS[k]
                nc.tensor.matmul(cp[:, 0:PCOLS],
                                 lhsT=convd[:, (i * 9 + k) * 128:(i * 9 + k + 1) * 128],
                                 rhs=xa[:, base:base + PCOLS],
                                 start=(k == 0), stop=(k == 8))
            pv4 = cp[:, 0:PCOLS].rearrange("p (f r w) -> p f r w",
                                           r=10, w=PADW)[:, :, 1:9, 1:9]
            zc4 = zc[:, c * ZCOLS:(c + 1) * ZCOLS].rearrange(
                "p (f h w) -> p f h w", h=8, w=8)
            nc.scalar.activation(zc4, pv4, ACTF.Copy,
                                 accum_out=sump_c[:, i * 16 + c:i * 16 + c + 1])
            sq = scr_p.tile([128, ZCOLS], F16, tag="sqcv", name=f"sqc{i}{c}")
            nc.vector.scalar_tensor_tensor(
                sq[:], zc[:, c * ZCOLS:(c + 1) * ZCOLS], 1.0,
                zc[:, c * ZCOLS:(c + 1) * ZCOLS], ALU.mult, ALU.mult,
                accum_out=sqp_c[:, i * 16 + c:i * 16 + c + 1])
        z_conv.append(zc)

        # x2-LIF trails by one tile so DVE never delays the next x1-LIF
        # (which gates the next conv matmul group on PE)
        if i >= 1:
            x2_lif(i - 1)
        # split the dw AllReduce so conv-LIF/gating of tiles 0-2 overlap the
        # conv matmuls of tiles 4-5
        if i == 2:
            emit_ar("dw", 0, 3)
            emit_params("dw", 0, 3)
        if i == 4:
            conv_lif_gate3(0)
            x2_lif(5)
        if i == 5:
            emit_ar("dw", 3, 6)
            emit_params("dw", 3, 6)
            conv_lif_gate3(3)
    for i in range(6):
        dump_rows("z_conv", 128 * i, z_conv[i])
    ctxE.close()
    if stop_after == 'conv':
        ctxZ.close(); ctxL.close(); return

    # ============ PHASE 6: fc2 + final residual ============
    ctxG = ExitStack()
    pG = ctxG.enter_context(tc.tile_pool(name="pG", bufs=1))
    wt_fc2 = []
    for i in range(6):
        w = pG.tile([128, C], F16, tag=f"wfc2_{i}", name=f"wfc2_{i}")
        nc.sync.dma_start(w[:], w_in["fc2"][128 * i:128 * (i + 1), :])
        wt_fc2.append(w)
    zf2 = []

    def zalloc_g(co):
        z = pG.tile([128, R], F16, tag=f"zf2{co}", name=f"zf2{co}")
        zf2.append(z)
        return z

    emit_linear("fc2", wt_fc2, gated, 0, 3, 6, zalloc_g, lambda co, z: None)
    emit_ar("fc2", 0, 3)
    emit_params("fc2", 0, 3)
    xms = []
    for pt in range(3):
        xm = pG.tile([128, R], F32, tag=f"xmr{pt}", name=f"xmr{pt}")
        nc.sync.dma_start(xm[:], xmid_sp[128 * pt:128 * (pt + 1), :])
        xms.append(xm)
    sos = [pG.tile([128, R], F16, tag=f"so{pt}", name=f"so{pt}", bufs=1)
           for pt in range(3)]
    for pt in range(3):
        dump_rows("z_fc2", 128 * pt, zf2[pt])
    emit_lif_multi([(zf2[pt], spike_writer(sos[pt]), "fc2", pt)
                    for pt in range(3)])
    for pt in range(3):
        xm = xms[pt]
        nc.vector.tensor_tensor(xm[:], xm[:], sos[pt][:], ALU.add)
        nc.sync.dma_start(out_d[128 * pt:128 * (pt + 1), :], xm[:])
    if tok_d is not None:
        tk = pG.tile([128, 1], F32, tag="tok", name="tk")
        nc.vector.memset(tk[:], 1.0)
        nc.sync.dma_start(tok_d[:], tk[:])
    ctxG.close()
    ctxZ.close()
    ctxL.close()


# ---------------- host glue ----------------

def _prep_inputs(inputs):
    x = np.asarray(inputs['x'], np.float32)
    xr = x.reshape(T, B, N, C)
    ident = np.eye(128, dtype=np.float16)
    mask = np.zeros((128, 512), np.float16)
    for blk in range(4):
        for h in range(4):
            mask[h * 32:(h + 1) * 32,
                 blk * 128 + h * 32:blk * 128 + (h + 1) * 32] = 0.125
    common = {"ident": ident, "mask": mask}
    for name in ("q", "k", "v", "p", "fc1", "fc2"):
        wdt = np.float16 if name in ("p", "fc2") else np.float32
        common[f"w_{name}"] = np.ascontiguousarray(
            np.asarray(inputs[name + "_w"]).T).astype(wdt)
    for name in ("q", "k", "v", "p", "fc1", "fc2"):
        common[f"pv_{name}"] = np.ascontiguousarray(np.stack(
            [np.asarray(inputs[name + "_g"], np.float32),
             np.asarray(inputs[name + "_be"], np.float32)], 1))
    common["pv_dw"] = np.ascontiguousarray(np.stack(
        [np.asarray(inputs["dw_g"], np.float32),
         np.asarray(inputs["dw_be"], np.float32)], 1))
    kv = np.asarray(inputs["dw_k"], np.float32).reshape(CH, 9)
    convd = np.zeros((54 * 128, 128), np.float16)
    for i in range(6):
        for k in range(9):
            blk = np.diag(kv[i * 128:(i + 1) * 128, k]).astype(np.float16)
            convd[(i * 9 + k) * 128:(i * 9 + k + 1) * 128, :] = blk
    common["convd"] = convd

    maps = []
    for c in range(NCORES):
        shard = xr[:, c * BS:(c + 1) * BS]
        xt = np.ascontiguousarray(shard.reshape(R, C).T)
        m = dict(common)
        m["xT"] = xt
        maps.append(m)
    return maps


_CACHE = {}


def _get_runner(debug_taps=False, timing=False, stop_after=None):
    key = (debug_taps, timing, stop_after)
    if key not in _CACHE:
        from runner_embed import SpmdRunner
        nc = build_kernel(debug_taps, timing, stop_after)
        _CACHE[key] = SpmdRunner(nc, NCORES)
    return _CACHE[key]


def kernel(**inputs):
    r = _get_runner()
    maps = _prep_inputs(inputs)
    args = r.prep(maps)
    outs = r.run(args)
    res = r.results(outs)
    full = np.empty((T, B, N, C), np.float32)
    for c in range(NCORES):
        o = res[c]["out"]
        full[:, c * BS:(c + 1) * BS] = o.T.reshape(T, BS, N, C)
    return np.ascontiguousarray(full.reshape(T * B, N, C))


# ---- embedded SPMD runner module ----
import types
runner_embed = types.ModuleType("runner_embed")
sys.modules["runner_embed"] = runner_embed
exec(r'''
import sys
sys.path.insert(0, '/opt/trn_rl_repo')
import numpy as np
import jax
from jax.sharding import Mesh, PartitionSpec, NamedSharding
from jax.experimental.shard_map import shard_map
import concourse.bass as bass
import concourse.mybir as mybir
from concourse.bass2jax import _bass_exec_p, install_neuronx_cc_hook, partition_id_tensor


class SpmdRunner:
    def __init__(self, nc, n_cores, repeat=1):
        install_neuronx_cc_hook()
        self.nc = nc
        self.n_cores = n_cores
        self.repeat = repeat
        partition_name = nc.partition_id_tensor.name if nc.partition_id_tensor else None
        in_names, out_names, out_avals, zero_outs = [], [], [], []
        for alloc in nc.m.functions[0].allocations:
            if not isinstance(alloc, mybir.MemoryLocationSet):
                continue
            name = alloc.memorylocations[0].name
            if alloc.kind == "ExternalInput":
                if name != partition_name:
                    in_names.append(name)
            elif alloc.kind == "ExternalOutput":
                shape = tuple(alloc.tensor_shape)
                dtype = mybir.dt.np(alloc.dtype)
                out_names.append(name)
                out_avals.append(jax.core.ShapedArray(shape, dtype))
                zero_outs.append(np.zeros(shape, dtype))
        self.in_names, self.out_names = in_names, out_names
        self.out_avals, self.zero_outs = out_avals, zero_outs
        n_params = len(in_names)
        n_outs = len(out_avals)
        all_in_names = list(in_names) + list(out_names)
        if partition_name is not None:
            all_in_names.append(partition_name)
        self.n_params = n_params

        nrep = self.repeat

        def _body(*args):
            operands = list(args)
            if partition_name is not None:
                operands.append(partition_id_tensor())
            all_outs = []
            for _ in range(nrep):
                outs = _bass_exec_p.bind(
                    *operands, out_avals=tuple(out_avals),
                    in_names=tuple(all_in_names), out_names=tuple(out_names),
                    lowering_input_output_aliases=(),
                    sim_require_finite=True, sim_require_nnan=True, nc=nc)
                all_outs.extend(outs)
                # chain: feed outputs back as the out-buffer operands of the
                # next call — defeats CSE and serializes the executions
                operands[n_params:n_params + n_outs] = list(outs)
            return tuple(all_outs)

        devices = jax.devices()[:n_cores]
        assert len(devices) == n_cores
        mesh = Mesh(np.asarray(devices), ("core",))
        self.mesh = mesh
        in_specs = (PartitionSpec("core"),) * (n_params + n_outs)
        out_specs = (PartitionSpec("core"),) * (n_outs * nrep)
        self.fn = jax.jit(
            shard_map(_body, mesh=mesh, in_specs=in_specs,
                      out_specs=out_specs, check_rep=False),
            keep_unused=True)

    def prep(self, in_maps):
        per_core = [[np.asarray(m[name]) for name in self.in_names]
                    for m in in_maps]
        concat_in = [np.concatenate([per_core[c][i] for c in range(self.n_cores)], axis=0)
                     for i in range(self.n_params)]
        concat_zeros = [np.zeros((self.n_cores * z.shape[0], *z.shape[1:]), z.dtype)
                        for z in self.zero_outs]
        sh = NamedSharding(self.mesh, PartitionSpec("core"))
        return [jax.device_put(a, sh) for a in concat_in + concat_zeros]

    def run(self, args):
        outs = self.fn(*args)
        jax.block_until_ready(outs)
        return outs

    def results(self, outs):
        res = []
        for c in range(self.n_cores):
            res.append({name: np.asarray(outs[i]).reshape(self.n_cores, *self.out_avals[i].shape)[c]
                        for i, name in enumerate(self.out_names)})
        return res

    def time_it(self, args, iters=20, warmup=3):
        import time
        for _ in range(warmup):
            self.run(args)
        ts = []
        for _ in range(iters):
            t0 = time.perf_counter()
            self.run(args)
            ts.append(time.perf_counter() - t0)
        ts = np.array(ts)
        return dict(min=ts.min(), median=float(np.median(ts)), mean=ts.mean())
''', runner_embed.__dict__)


# revision 46
# speedup vs baseline: 1.1412x; 1.1412x over previous
"""Trainium2 Bass kernel for nn_Block_76519137345684 (Spikformer-style block:
spiking self-attention + spiking gated MLP with training-mode BatchNorm).

Strategy
- Data-parallel over batch B across 8 NeuronCores (16 batch each). BN batch
  statistics (per-channel sum / sum-of-squares) are AllReduced across cores.
- Activations live channel-on-partition: (C, rows) with rows
  r = ((t*16 + b)*64 + n); LIF timesteps are contiguous 1024-column slices.
- BN application is FUSED into the LIF recurrence on DVE in f16:
      yh_t = A*z_t + C   (per-partition A=0.5*a, C=0.5*c as tensor_scalar
                          AP operands - scalar APs don't break DVE fast modes)
      u_t  = 0.5*u_{t-1}*[u_{t-1} < thr] + yh_t,  s_t = [u_t >= thr]
- Attention uses associativity: y = q @ (k^T v) * scale; per-head block
  structure enforced with a 0.125-scaled block-diagonal mask.
- Depthwise 3x3 conv on the TENSOR engine: spikes written into a zero-padded
  plane (10 rows x 10 cols per frame); 9 taps = 9 PSUM-accumulated matmuls
  with diagonal per-channel weights against shifted plane views. PSUM chunks
  are 4 frames (400 cols) and evacuate DIRECTLY to the contiguous z layout
  via strided ACT copy with fused stats accumulation.
- Matmul dtypes: f32r for continuous-input layers (q,k,v,fc1), fp16 for
  binary-input layers (p, fc2, conv) and attention.
- xmid (residual) stays in SBUF through fc1; spilled to DRAM only for the
  final residual read.
"""
import sys
sys.path.insert(0, '/opt/trn_rl_repo')
import numpy as np

import concourse.bass as bass
import concourse.mybir as mybir
import concourse.tile as tile
from concourse.tile import add_dep_helper

T, B, N, C = 4, 128, 64, 384
HID, CH, HEADS, HD = 1536, 768, 12, 32
NCORES = 8
BS = B // NCORES
R = T * BS * N              # 4096 rows per core
TC = BS * N                 # 1024 cols per timestep
COUNT = T * B * N           # 32768 rows globally (BN stat count)
EPS = 1e-5
PADW = 10
PADP = PADW * 10            # 100 per frame plane
NFR = T * BS                # 64 frames
GUARD = 16
PLANE = NFR * PADP          # 6400
PADL = GUARD + PLANE + GUARD
FR_CH = 4                   # frames per conv psum chunk
CCH = NFR // FR_CH          # 16 chunks per tile
PCOLS = FR_CH * PADP        # 400 plane cols per chunk
ZCOLS = FR_CH * 64          # 256 z cols per chunk

F32 = mybir.dt.float32
F32R = mybir.dt.float32r
F16 = mybir.dt.float16
ALU = mybir.AluOpType
ACTF = mybir.ActivationFunctionType

_ctr = [0]


def _fix_multiwaits(nc):
    """walrus here accepts max 1 sync-wait per instruction: split extras
    onto same-engine NOPs."""
    for f in nc.m.functions:
        for bb in f.blocks:
            new_insts = []
            for inst in bb.instructions:
                si = inst.sync_info
                ow = list(si.on_wait) if (si and si.on_wait) else []
                if len(ow) > 1:
                    for w in ow[:-1]:
                        _ctr[0] += 1
                        new_insts.append(mybir.InstNoOp(
                            name=f"I-waitnop-{_ctr[0]}", engine=inst.engine,
                            sync_info=mybir.SyncInfo(on_wait=[w], on_update=[]),
                            bass_nofuse=True))
                    si.on_wait = [ow[-1]]
                new_insts.append(inst)
            bb.instructions[:] = new_insts


def build_kernel(debug_taps=False, timing=False, stop_after=None):
    nc = bass.Bass("TRN2", target_bir_lowering=False, debug=False,
                   num_devices=NCORES)

    xT_in = nc.declare_dram_parameter("xT", [C, R], F32R, isOutput=False)
    w_in = {}
    for name, ci, co, dt in [("q", C, C, F32R), ("k", C, C, F32R),
                             ("v", C, C, F32R), ("p", C, C, F16),
                             ("fc1", C, HID, F32R), ("fc2", CH, C, F16)]:
        w_in[name] = nc.declare_dram_parameter(f"w_{name}", [ci, co], dt,
                                               isOutput=False)
    pv_in = {}
    for name, co in [("q", C), ("k", C), ("v", C), ("p", C),
                     ("fc1", HID), ("dw", CH), ("fc2", C)]:
        pv_in[name] = nc.declare_dram_parameter(f"pv_{name}", [co, 2], F32,
                                                isOutput=False)
    ident_in = nc.declare_dram_parameter("ident", [128, 128], F16, isOutput=False)
    mask_in = nc.declare_dram_parameter("mask", [128, 512], F16, isOutput=False)
    # 54 diagonal [128,128] f16 weight matrices: (tile i, tap k) at row
    # (i*9+k)*128
    convd_in = nc.declare_dram_parameter("convd", [54 * 128, 128], F16,
                                         isOutput=False)
    if timing:
        out_d = nc.dram_tensor("out", [C, R], F32)
        tok_d = nc.declare_dram_parameter("tok", [128, 1], F32, isOutput=True)
    else:
        out_d = nc.declare_dram_parameter("out", [C, R], F32, isOutput=True)
        tok_d = None

    dbg = {}
    if debug_taps:
        for nm, npt, dt in [("z_q", 3, F16), ("s_q", 3, F16), ("s_k", 3, F16),
                            ("s_v", 3, F16), ("z_y", 3, F16), ("s_y", 3, F16),
                            ("z_p", 3, F16), ("xmid", 3, F32),
                            ("z_fc1", 12, F16), ("z_conv", 6, F16),
                            ("s_conv", 6, F16), ("gated", 6, F16),
                            ("z_fc2", 3, F16)]:
            dbg[nm] = nc.declare_dram_parameter(f"dbg_{nm}", [npt * 128, R],
                                                dt, isOutput=True)

    cc = {}
    for name, co in [("qk", 2 * C), ("v", C), ("p", C),
                     ("fc1", HID), ("dw", CH), ("fc2", C)]:
        cci = nc.dram_tensor(f"cci_{name}", [co, 2], F32)
        cco = nc.dram_tensor(f"cco_{name}", [co, 2], F32, addr_space="Shared")
        cc[name] = (cci, cco)

    xmid_sp = nc.dram_tensor("xmid_spill", [C, R], F32)
    zx2_sp = nc.dram_tensor("zx2_spill", [CH, R], F16)

    with tile.TileContext(nc, pool_alloc_mode="queue") as tc:
        _body(nc, tc, xT_in, w_in, pv_in, ident_in, mask_in, convd_in, out_d,
              tok_d, cc, xmid_sp, zx2_sp, dbg, stop_after)
    _fix_multiwaits(nc)
    return nc


def _body(nc, tc, xT_in, w_in, pv_in, ident_in, mask_in, convd_in, out_d,
          tok_d, cc, xmid_sp, zx2_sp, dbg, stop_after=None):
    from contextlib import ExitStack

    # spike DRAM buffers (cross-phase hand-off)
    s_d = {name: nc.dram_tensor(f"s{name}_d", [C, R], F16)
           for name in ("q", "k", "v", "y")}

    # ---------- long-lived small pools ----------
    ctxL = ExitStack()
    const_p = ctxL.enter_context(tc.tile_pool(name="const", bufs=1))
    stat_p = ctxL.enter_context(tc.tile_pool(name="stats", bufs=1))
    scr_p = ctxL.enter_context(tc.tile_pool(name="scr", bufs=2))
    lif_p = ctxL.enter_context(tc.tile_pool(name="lifp", bufs=1))
    ps_mm = ctxL.enter_context(tc.tile_pool(name="psmm", bufs=4, space="PSUM"))
    ps_at = ctxL.enter_context(tc.tile_pool(name="psat", bufs=1, space="PSUM"))

    ident = const_p.tile([128, 128], F16, tag="ident", name="ident")
    nc.sync.dma_start(ident[:], ident_in[:])
    mask = const_p.tile([128, 512], F16, tag="mask", name="mask")
    nc.sync.dma_start(mask[:], mask_in[:])
    pvec = {}
    for name, npt in [("q", 3), ("k", 3), ("v", 3), ("p", 3),
                      ("fc1", 12), ("dw", 6), ("fc2", 3)]:
        pv = const_p.tile([128, 2 * npt], F32, tag=f"pv_{name}",
                          name=f"pv_{name}")
        for i in range(npt):
            nc.sync.dma_start(pv[:, 2 * i:2 * i + 2],
                              pv_in[name][128 * i:128 * (i + 1), :])
        pvec[name] = pv

    # stats: sump, sqp [128, ngrp*npt]; stfin [128, 2*npt] blocked (S | Q);
    # AC [128, 2*npt] blocked (A | C)
    NGRP = {"q": 8, "k": 8, "v": 8, "p": 8, "fc1": 8, "fc2": 8, "dw": 16}
    NPT = {"q": 3, "k": 3, "v": 3, "p": 3, "fc1": 12, "dw": 6, "fc2": 3}
    STT = {}
    for name, npt in [("q", 3), ("k", 3), ("v", 3), ("p", 3),
                      ("fc1", 12), ("dw", 6), ("fc2", 3)]:
        g = NGRP[name]
        STT[name] = (
            stat_p.tile([128, g * npt], F32, tag=f"sum_{name}", name=f"sum_{name}"),
            stat_p.tile([128, g * npt], F32, tag=f"sq_{name}", name=f"sq_{name}"),
            stat_p.tile([128, 2 * npt], F32, tag=f"st_{name}", name=f"st_{name}"),
            stat_p.tile([128, 2 * npt], F32, tag=f"ac_{name}", name=f"ac_{name}"),
        )

    # ---------- helpers ----------
    def emit_linear(name, wt, rhs, co_lo, co_hi, n_ci, z_alloc, z_done):
        """z tiles are f16. PSUM evac split between ACT and DVE per chunk so
        neither engine starves the PE; sq-sums on the other engine."""
        sump, sqp = STT[name][0], STT[name][1]
        for co in range(co_lo, co_hi):
            z = z_alloc(co)
            for cg in range(2):
                pss = [ps_mm.tile([128, 512], F32, tag="ps", name=f"ps{name}{co}{cg}{j}")
                       for j in range(4)]
                for ci in range(n_ci):
                    for ch in range(4):
                        nc.tensor.matmul(
                            pss[ch][:],
                            lhsT=wt[ci][:, co * 128:(co + 1) * 128],
                            rhs=rhs[ci][:, (cg * 4 + ch) * 512:(cg * 4 + ch + 1) * 512],
                            start=(ci == 0), stop=(ci == n_ci - 1))
                for ch in range(4):
                    g = cg * 4 + ch
                    sl = slice(g * 512, (g + 1) * 512)
                    sq = scr_p.tile([128, 512], F16, tag="sqscr", name="sqscr")
                    if g % 2 == 0:
                        nc.scalar.activation(z[:, sl], pss[ch][:], ACTF.Copy,
                                             accum_out=sump[:, co * 8 + g:co * 8 + g + 1])
                        nc.vector.scalar_tensor_tensor(
                            sq[:], z[:, sl], 1.0, z[:, sl], ALU.mult, ALU.mult,
                            accum_out=sqp[:, co * 8 + g:co * 8 + g + 1])
                    else:
                        nc.vector.tensor_scalar(
                            z[:, sl], pss[ch][:], 1.0, 0.0, ALU.mult, ALU.add,
                            accum_out=sump[:, co * 8 + g:co * 8 + g + 1])
                        nc.scalar.activation(
                            sq[:], pss[ch][:], ACTF.Square,
                            accum_out=sqp[:, co * 8 + g:co * 8 + g + 1])
            z_done(co, z)

    def _ar_reduce_in(name, lo, hi, cci, row0, dmas):
        """tensor_reduce per-ptile stats into stfin (blocked S|Q) and DMA
        them into `cci` starting at block row0. Stats DMAs ride the idle
        GpSimd queue so they don't wait behind bulk spills on Sync."""
        ngrp = NGRP[name]
        npt = NPT[name]
        sump, sqp, stfin, _ = STT[name]
        for co in range(lo, hi):
            nc.vector.tensor_reduce(stfin[:, co:co + 1],
                                    sump[:, co * ngrp:(co + 1) * ngrp],
                                    axis=mybir.AxisListType.X, op=ALU.add)
            nc.vector.tensor_reduce(stfin[:, npt + co:npt + co + 1],
                                    sqp[:, co * ngrp:(co + 1) * ngrp],
                                    axis=mybir.AxisListType.X, op=ALU.add)
            r = 128 * (row0 + co - lo)
            dmas.append(nc.gpsimd.dma_start(cci[r:r + 128, 0:1],
                                            stfin[:, co:co + 1]))
            dmas.append(nc.gpsimd.dma_start(cci[r:r + 128, 1:2],
                                            stfin[:, npt + co:npt + co + 1]))

    def _ar_read_back(name, lo, hi, cco, row0):
        npt = NPT[name]
        stfin = STT[name][2]
        outs = []
        for co in range(lo, hi):
            r = 128 * (row0 + co - lo)
            outs.append(nc.gpsimd.dma_start(stfin[:, co:co + 1],
                                            cco[r:r + 128, 0:1]))
            outs.append(nc.gpsimd.dma_start(stfin[:, npt + co:npt + co + 1],
                                            cco[r:r + 128, 1:2]))
        return outs

    def emit_ar(name, lo, hi, key=None):
        """AllReduce stats for ptiles [lo, hi) of `name`."""
        cci, cco = cc[key or name]
        dmas = []
        _ar_reduce_in(name, lo, hi, cci, lo, dmas)
        ar = nc.gpsimd.collective_compute(
            "AllReduce", ALU.add, replica_groups=[list(range(NCORES))],
            ins=[cci[128 * lo:128 * hi, :]], outs=[cco[128 * lo:128 * hi, :]])
        for d in dmas:
            add_dep_helper(ar.ins, d.ins, reason="ar waits dma_in")
        for d in _ar_read_back(name, lo, hi, cco, lo):
            add_dep_helper(d.ins, ar.ins, reason="readback waits ar")

    def emit_ar_names(names, key):
        """ONE AllReduce covering several names' full stats — avoids the
        per-collective wakeup/stagger on the CC core."""
        cci, cco = cc[key]
        dmas = []
        row0 = 0
        for name in names:
            _ar_reduce_in(name, 0, NPT[name], cci, row0, dmas)
            row0 += NPT[name]
        ar = nc.gpsimd.collective_compute(
            "AllReduce", ALU.add, replica_groups=[list(range(NCORES))],
            ins=[cci[:]], outs=[cco[:]])
        for d in dmas:
            add_dep_helper(ar.ins, d.ins, reason="ar waits dma_in")
        row0 = 0
        for name in names:
            for d in _ar_read_back(name, 0, NPT[name], cco, row0):
                add_dep_helper(d.ins, ar.ins, reason="readback waits ar")
            row0 += NPT[name]

    def emit_params(name, lo, hi):
        """Batched: compute A = 0.5*a into AC[:, lo:hi] and C = 0.5*c into
        AC[:, npt+lo:npt+hi] with [128, n]-wide ops."""
        npt = NPT[name]
        _, _, stfin, AC = STT[name]
        pv = pvec[name]
        n = hi - lo
        S = stfin[:, lo:hi]
        Q = stfin[:, npt + lo:npt + hi]
        pvv = pv[:, 2 * lo:2 * hi].rearrange("p (n t) -> p t n", t=2)
        g_ = pvv[:, 0, :]
        be_ = pvv[:, 1, :]
        w = scr_p.tile([128, 6 * n], F32, tag="pscr", name="pscr")
        mean, qm, var, sd, inv, a_ = (w[:, j * n:(j + 1) * n] for j in range(6))
        nc.vector.tensor_scalar(mean, S, 1.0 / COUNT, None, ALU.mult)
        nc.vector.tensor_scalar(qm, Q, 1.0 / COUNT, None, ALU.mult)
        nc.vector.tensor_tensor(var, mean, mean, ALU.mult)
        nc.vector.tensor_tensor(var, qm, var, ALU.subtract)
        nc.vector.tensor_scalar(var, var, EPS, None, ALU.add)
        nc.scalar.sqrt(sd, var)
        nc.vector.reciprocal(inv, sd)
        nc.vector.tensor_tensor(a_, g_, inv, ALU.mult)
        nc.vector.tensor_scalar(AC[:, lo:hi], a_, 0.5, None, ALU.mult)
        # C = 0.5*(be - mean*a)
        nc.vector.tensor_tensor(qm, mean, a_, ALU.mult)
        nc.vector.tensor_tensor(qm, be_, qm, ALU.subtract)
        nc.vector.tensor_scalar(AC[:, npt + lo:npt + hi], qm, 0.5, None,
                                ALU.mult)

    def emit_lif_multi(items, thr=1.0):
        """Fused norm+LIF over a GROUP of z tiles, ops interleaved across
        tiles so dependent DVE ops don't stall the pipeline back-to-back.
        items: list of (z, writer, name, pt). The BN affine yh_t = A*z_t + C
        runs on ACT; the recurrence on DVE.
        u_t = 0.5*u_{t-1}*[u_{t-1} < thr] + yh_t ; writer(t, u_ap) emits
        spikes."""
        n = len(items)
        us, scs, hs, y2s = [], [], [], []
        for j in range(n):
            us.append(lif_p.tile([128, TC], F16, tag=f"lifu{j}",
                                 name=f"lifu{j}", bufs=1))
            scs.append(lif_p.tile([128, TC], F16, tag=f"lifsc{j}",
                                  name=f"lifsc{j}", bufs=1))
            hs.append(lif_p.tile([128, TC], F16, tag=f"lifh{j}",
                                 name=f"lifh{j}", bufs=1))
            y2s.append(lif_p.tile([128, TC], F16, tag=f"lify{j}",
                                  name=f"lify{j}", bufs=1))

        def yh_chunk(j, dst, t):
            z, _, name, pt = items[j]
            if name is None:
                return z[:, t * TC:(t + 1) * TC]
            AC = STT[name][3]
            npt = NPT[name]
            nc.scalar.activation(dst[:], z[:, t * TC:(t + 1) * TC],
                                 ACTF.Identity, scale=AC[:, pt:pt + 1],
                                 bias=AC[:, npt + pt:npt + pt + 1])
            return dst[:]

        ucur = [yh_chunk(j, us[j], 0) for j in range(n)]
        for t in range(T):
            for j in range(n):
                items[j][1](t, ucur[j])
            if t < T - 1:
                for j in range(n):
                    nc.vector.tensor_scalar(scs[j][:], ucur[j], thr, 0.5,
                                            ALU.is_lt, ALU.mult)
                yn = [yh_chunk(j, y2s[j], t + 1) for j in range(n)]
                for j in range(n):
                    nc.vector.tensor_tensor(hs[j][:], ucur[j], scs[j][:],
                                            ALU.mult)
                for j in range(n):
                    nc.vector.tensor_tensor(us[j][:], hs[j][:], yn[j],
                                            ALU.add)
                    ucur[j] = us[j][:]

    def emit_lif(z, writer, name=None, pt=0, thr=1.0):
        emit_lif_multi([(z, writer, name, pt)], thr=thr)

    def spike_writer(st, thr=1.0, eng=None):
        e = eng if eng is not None else nc.vector
        def w(t, ucur, st=st, thr=thr, e=e):
            e.tensor_scalar(st[:, t * TC:(t + 1) * TC], ucur, thr, None,
                            ALU.is_ge)
        return w

    def dump_rows(nm, row0, t_):
        if nm in dbg:
            nc.sync.dma_start(dbg[nm][row0:row0 + 128, :], t_[:])

    # ============ PHASE 1: q,k,v matmul + AR + LIF -> spikes to DRAM ======
    ctxA = ExitStack()
    pA = ctxA.enter_context(tc.tile_pool(name="pA", bufs=1))
    wts = {}
    for name in ("q", "k", "v"):
        wt = []
        for i in range(3):
            w = pA.tile([128, C], F32R, tag=f"w_{name}{i}", name=f"w_{name}{i}")
            nc.sync.dma_start(w[:], w_in[name][128 * i:128 * (i + 1), :])
            wt.append(w)
        wts[name] = wt
    xT = []
    for i in range(3):
        x = pA.tile([128, R], F32R, tag=f"xT{i}", name=f"xT{i}")
        for c4 in range(4):
            nc.sync.dma_start(x[:, 1024 * c4:1024 * (c4 + 1)],
                              xT_in[128 * i:128 * (i + 1),
                                    1024 * c4:1024 * (c4 + 1)])
        xT.append(x)

    zs = {}
    for name in ("q", "k", "v"):
        zt = []

        def zalloc(co, name=name, zt=zt):
            z = pA.tile([128, R], F16, tag=f"z{name}{co}", name=f"z{name}{co}",
                        bufs=1)
            zt.append(z)
            return z

        emit_linear(name, wts[name], xT, 0, 3, 3, zalloc, lambda co, z: None)
        zs[name] = zt
        if name == "k":
            # one AR for q+k stats, hidden under v's matmuls
            emit_ar_names(("q", "k"), "qk")
            emit_params("q", 0, 3)
            emit_params("k", 0, 3)
    emit_ar("v", 0, 3)
    emit_params("v", 0, 3)
    for pt in range(3):
        dump_rows("z_q", 128 * pt, zs["q"][pt])

    # LIF order k, v first so attention transposes can start while q runs;
    # each name's 3 ptiles run interleaved to keep the DVE pipe full
    for name in ("k", "v", "q"):
        sts = [pA.tile([128, R], F16, tag=f"spt{pt}", name=f"s{name}{pt}",
                       bufs=1) for pt in range(3)]
        emit_lif_multi([(zs[name][pt], spike_writer(sts[pt]), name, pt)
                        for pt in range(3)])
        for pt in range(3):
            nc.sync.dma_start(s_d[name][128 * pt:128 * (pt + 1), :],
                              sts[pt][:])
            dump_rows(f"s_{name}", 128 * pt, sts[pt])
    ctxA.close()
    if stop_after == 'qkv':
        ctxL.close(); return

    # ============ PHASE 2: transposes + attention + y-LIF ============
    ctxB = ExitStack()
    pB = ctxB.enter_context(tc.tile_pool(name="pB", bufs=1))
    # rm layout: per (pt, tb) a (128, 128) block at col (pt*64+tb)*128;
    # rows 0..63 = transposed spikes (n-major), rows 64..127 stay ZERO so
    # mm1 can contract over the full K=128 (K=64 matmuls hang on this HW).
    rm = {}
    for name in ("k", "v"):
        rmt = pB.tile([128, 6 * R], F16, tag=f"rm_{name}", name=f"rm_{name}")
        nc.gpsimd.memset(rmt[64:128, :], 0.0)
        for pt in range(3):
            srt = pB.tile([128, R], F16, tag=f"skvr{pt % 2}",
                          name=f"r{name}{pt}")
            nc.sync.dma_start(srt[:], s_d[name][128 * pt:128 * (pt + 1), :])
            for grp in range(8):
                pst = ps_at.tile([128, 1024], F16, tag="pstr", name="pstr")
                for j in range(8):
                    tb = grp * 8 + j
                    nc.tensor.transpose(pst[0:64, 128 * j:128 * (j + 1)],
                                        srt[:, 64 * tb:64 * (tb + 1)],
                                        ident[:])
                nc.scalar.copy(
                    rmt[0:64, (pt * 64 + grp * 8) * 128:(pt * 64 + grp * 8 + 8) * 128],
                    pst[0:64, :])
        rm[name] = rmt

    sy = []
    for pt in range(3):
        sqr = pB.tile([128, R], F16, tag=f"sqr{pt % 2}", name=f"sqr{pt}")
        nc.sync.dma_start(sqr[:], s_d["q"][128 * pt:128 * (pt + 1), :])
        zy = pB.tile([128, R], F16, tag=f"zy{pt % 2}", name=f"zy{pt}")
        for g4 in range(16):
            mm1ps = ps_at.tile([128, 512], F32, tag="mm1", name="mm1")
            for j in range(4):
                tb = g4 * 4 + j
                base = (pt * 64 + tb) * 128
                nc.tensor.matmul(mm1ps[:, 128 * j:128 * (j + 1)],
                                 lhsT=rm["k"][:, base:base + 128],
                                 rhs=rm["v"][:, base:base + 128],
                                 start=True, stop=True)
            m4 = scr_p.tile([128, 512], F16, tag="m4", name="m4")
            nc.vector.tensor_tensor(m4[:], mm1ps[:], mask[:], ALU.mult)
            yps = ps_at.tile([128, 256], F32, tag="yps", name="yps", bufs=2)
            for j in range(4):
                tb = g4 * 4 + j
                nc.tensor.matmul(yps[:, 64 * j:64 * (j + 1)],
                                 lhsT=m4[:, 128 * j:128 * (j + 1)],
                                 rhs=sqr[:, 64 * tb:64 * (tb + 1)],
                                 start=True, stop=True)
            # evacuate with 0.5 scale: zy holds Y = 0.5 * z_y
            nc.scalar.activation(zy[:, 256 * g4:256 * (g4 + 1)], yps[:],
                                 ACTF.Copy, scale=0.5)
        dump_rows("z_y", 128 * pt, zy)
        syt = pB.tile([128, R], F16, tag=f"sy{pt % 2}", name=f"sy{pt}")
        emit_lif(zy, spike_writer(syt, thr=0.5), thr=0.5)
        nc.sync.dma_start(s_d["y"][128 * pt:128 * (pt + 1), :], syt[:])
        dump_rows("s_y", 128 * pt, syt)
    ctxB.close()
    if stop_after == 'attn':
        ctxL.close(); return

    # ============ PHASE 3: p-linear + xmid (xr stays in SBUF for fc1) =====
    ctxZ = ExitStack()
    pZ = ctxZ.enter_context(tc.tile_pool(name="pZ", bufs=1))
    ctxC2 = ExitStack()
    pC2 = ctxC2.enter_context(tc.tile_pool(name="pC2", bufs=1))
    ctxC1 = ExitStack()
    pC1 = ctxC1.enter_context(tc.tile_pool(name="pC1", bufs=1))
    syr = []
    for i in range(3):
        s = pC1.tile([128, R], F16, tag=f"syr{i}", name=f"syr{i}")
        nc.sync.dma_start(s[:], s_d["y"][128 * i:128 * (i + 1), :])
        syr.append(s)
    wt_p = []
    for i in range(3):
        w = pC1.tile([128, C], F16, tag=f"w_p{i}", name=f"w_p{i}")
        nc.sync.dma_start(w[:], w_in["p"][128 * i:128 * (i + 1), :])
        wt_p.append(w)
    zp = []

    def zalloc_p(co):
        z = pC1.tile([128, R], F16, tag=f"zp{co}", name=f"zp{co}")
        zp.append(z)
        return z

    emit_linear("p", wt_p, syr, 0, 3, 3, zalloc_p, lambda co, z: None)
    emit_ar("p", 0, 3)
    emit_params("p", 0, 3)
    spts = [pC1.tile([128, R], F16, tag=f"sptr{pt}", name=f"sp{pt}", bufs=1)
            for pt in range(3)]
    # xr loads issued before the LIF so they fully overlap it
    # (xr written as f32r so the fc1 f32r matmul can consume it directly)
    xr_t = []
    for pt in range(3):
        xr = pC2.tile([128, R], F32R, tag=f"xm{pt}", name=f"xm{pt}")
        nc.sync.dma_start(xr[:], xT_in[128 * pt:128 * (pt + 1), :])
        xr_t.append(xr)
    for pt in range(3):
        dump_rows("z_p", 128 * pt, zp[pt])
    emit_lif_multi([(zp[pt], spike_writer(spts[pt]), "p", pt)
                    for pt in range(3)])
    for pt in range(3):
        # xr = x + p_spikes ; kept in SBUF for fc1, spilled for final residual
        xr = xr_t[pt]
        nc.vector.tensor_tensor(xr[:], xr[:].bitcast(F32), spts[pt][:],
                                ALU.add)
        nc.sync.dma_start(xmid_sp[128 * pt:128 * (pt + 1), :],
                          xr[:].bitcast(F32))
        dump_rows("xmid", 128 * pt, xr[:].bitcast(F32))
    ctxC1.close()
    if stop_after == 'p':
        ctxC2.close(); ctxZ.close(); ctxL.close(); return

    # ============ PHASE 4: fc1 (z tiles stay in SBUF) ============
    ctxD = ExitStack()
    pD = ctxD.enter_context(tc.tile_pool(name="pD", bufs=1))
    wt_fc1 = []
    for i in range(3):
        w = pD.tile([128, HID], F32R, tag=f"wfc1_{i}", name=f"wfc1_{i}")
        nc.sync.dma_start(w[:], w_in["fc1"][128 * i:128 * (i + 1), :])
        wt_fc1.append(w)
    xmid_v = [x[:] for x in xr_t]

    zf1 = {}

    def zalloc_f(co):
        if co < 6:
            z = pZ.tile([128, R], F16, tag=f"zf1_{co}", name=f"zf1_{co}")
            zf1[co] = z
        else:
            z = pD.tile([128, R], F16, tag=f"zx2_{co % 2}", name=f"zf1_{co}",
                        bufs=2)
        return z

    def zdone_f(co, z):
        if co >= 6:
            nc.sync.dma_start(zx2_sp[128 * (co - 6):128 * (co - 5), :], z[:])
        if "z_fc1" in dbg:
            nc.sync.dma_start(dbg["z_fc1"][128 * co:128 * (co + 1), :], z[:])

    emit_linear("fc1", wt_fc1, xmid_v, 0, 12, 3, zalloc_f, zdone_f)
    emit_ar("fc1", 0, 12)
    emit_params("fc1", 0, 12)
    ctxD.close()
    ctxC2.close()
    if stop_after == 'fc1':
        ctxZ.close(); ctxL.close(); return

    # ============ PHASE 5a: x1-LIF -> spike planes -> PE conv ============
    ctxE = ExitStack()
    pE = ctxE.enter_context(tc.tile_pool(name="pE", bufs=1))
    convd = pE.tile([128, 54 * 128], F16, tag="convd", name="convd")
    for i in range(54):
        nc.sync.dma_start(convd[:, 128 * i:128 * (i + 1)],
                          convd_in[128 * i:128 * (i + 1), :])
    # tap shift offsets in plane space, kh-major to match host convd order
    SHIFTS = [dh * PADW + dw for dh in (-1, 0, 1) for dw in (-1, 0, 1)]

    z_conv = []
    sx2_t = []
    gated = [None] * 6
    sump_c, sqp_c, _, _ = STT["dw"]

    def conv_lif_gate3(lo):
        scvs = [pE.tile([128, R], F16, tag=f"scv{j % 3}", name=f"scv{j}",
                        bufs=1) for j in range(lo, lo + 3)]
        emit_lif_multi([(z_conv[j], spike_writer(scvs[j - lo]), "dw", j)
                        for j in range(lo, lo + 3)])
        for j in range(lo, lo + 3):
            dump_rows("s_conv", 128 * j, scvs[j - lo])
            g = pZ.tile([128, R], F16, tag=f"zf1_{j}", name=f"gated{j}")
            nc.vector.tensor_tensor(g[:], scvs[j - lo][:], sx2_t[j][:],
                                    ALU.mult)
            gated[j] = g
            dump_rows("gated", 128 * j, g)

    def x2_lif(i):
        zx2 = pE.tile([128, R], F16, tag="zx2r", name=f"zx2r{i}", bufs=2)
        nc.sync.dma_start(zx2[:], zx2_sp[128 * i:128 * (i + 1), :])
        sx2 = pE.tile([128, R], F16, tag=f"gt{i % 5}", name=f"sx2_{i}")
        emit_lif(zx2, spike_writer(sx2), name="fc1", pt=6 + i)
        sx2_t.append(sx2)

    for i in range(6):
        xa = pE.tile([128, PADL], F16, tag=f"cxa{i % 2}", name=f"cxa{i}")
        if i < 2:
            nc.gpsimd.memset(xa[:], 0.0)

        def x1_writer(t, ucur, xa=xa):
            # one strided is_ge into the padded plane per timestep
            xa4 = xa[:, GUARD + t * BS * PADP:GUARD + (t + 1) * BS * PADP] \
                .rearrange("p (f r w) -> p f r w", r=10, w=PADW)[:, :, 1:9, 1:9]
            u4 = ucur.rearrange("p (f h w) -> p f h w", h=8, w=8)
            nc.vector.tensor_scalar(xa4, u4, 1.0, None, ALU.is_ge)

        emit_lif(zf1[i], x1_writer, name="fc1", pt=i)

        # 9-tap depthwise conv via diagonal-weight matmuls; psum chunks of
        # FR_CH frames evacuate directly to contiguous z layout with stats
        zc = pZ.tile([128, R], F16, tag=f"zf1_{i}", name=f"zconv{i}")
        for c in range(CCH):
            cp = ps_mm.tile([128, 512], F32, tag="ps", name=f"cps{i}{c}")
            for k in range(9):
                base = GUARD + c * PCOLS + SHIFTS[k]
                nc.tensor.matmul(cp[:, 0:PCOLS],
                                 lhsT=convd[:, (i * 9 + k) * 128:(i * 9 + k + 1) * 128],
                                 rhs=xa[:, base:base + PCOLS],
                                 start=(k == 0), stop=(k == 8))
            pv4 = cp[:, 0:PCOLS].rearrange("p (f r w) -> p f r w",
                                           r=10, w=PADW)[:, :, 1:9, 1:9]
            zc4 = zc[:, c * ZCOLS:(c + 1) * ZCOLS].rearrange(
                "p (f h w) -> p f h w", h=8, w=8)
            nc.scalar.activation(zc4, pv4, ACTF.Copy,
                                 accum_out=sump_c[:, i * 16 + c:i * 16 + c + 1])
            sq = scr_p.tile([128, ZCOLS], F16, tag="sqcv", name=f"sqc{i}{c}")
            nc.vector.scalar_tensor_tensor(
                sq[:], zc[:, c * ZCOLS:(c + 1) * ZCOLS], 1.0,
                zc[:, c * ZCOLS:(c + 1) * ZCOLS], ALU.mult, ALU.mult,
                accum_out=sqp_c[:, i * 16 + c:i * 16 + c + 1])
        z_conv.append(zc)

        # x2-LIF trails by one tile so DVE never delays the next x1-LIF
        # (which gates the next conv matmul group on PE)
        if i >= 1:
            x2_lif(i - 1)
        # split the dw AllReduce so conv-LIF/gating of tiles 0-2 overlap the
        # conv matmuls of tiles 4-5
        if i == 2:
            emit_ar("dw", 0, 3)
            emit_params("dw", 0, 3)
        if i == 4:
            conv_lif_gate3(0)
        if i == 5:
            x2_lif(5)
            emit_ar("dw", 3, 6)
            emit_params("dw", 3, 6)
            conv_lif_gate3(3)
    for i in range(6):
        dump_rows("z_conv", 128 * i, z_conv[i])
    ctxE.close()
    if stop_after == 'conv':
        ctxZ.close(); ctxL.close(); return

    # ============ PHASE 6: fc2 + final residual ============
    ctxG = ExitStack()
    pG = ctxG.enter_context(tc.tile_pool(name="pG", bufs=1))
    wt_fc2 = []
    for i in range(6):
        w = pG.tile([128, C], F16, tag=f"wfc2_{i}", name=f"wfc2_{i}")
        nc.sync.dma_start(w[:], w_in["fc2"][128 * i:128 * (i + 1), :])
        wt_fc2.append(w)
    zf2 = []

    def zalloc_g(co):
        z = pG.tile([128, R], F16, tag=f"zf2{co}", name=f"zf2{co}")
        zf2.append(z)
        return z

    emit_linear("fc2", wt_fc2, gated, 0, 3, 6, zalloc_g, lambda co, z: None)
    emit_ar("fc2", 0, 3)
    emit_params("fc2", 0, 3)
    xms = []
    for pt in range(3):
        xm = pG.tile([128, R], F32, tag=f"xmr{pt}", name=f"xmr{pt}")
        nc.sync.dma_start(xm[:], xmid_sp[128 * pt:128 * (pt + 1), :])
        xms.append(xm)
    sos = [pG.tile([128, R], F16, tag=f"so{pt}", name=f"so{pt}", bufs=1)
           for pt in range(3)]
    for pt in range(3):
        dump_rows("z_fc2", 128 * pt, zf2[pt])
    emit_lif_multi([(zf2[pt], spike_writer(sos[pt]), "fc2", pt)
                    for pt in range(3)])
    for pt in range(3):
        xm = xms[pt]
        nc.vector.tensor_tensor(xm[:], xm[:], sos[pt][:], ALU.add)
        nc.sync.dma_start(out_d[128 * pt:128 * (pt + 1), :], xm[:])
    if tok_d is not None:
        tk = pG.tile([128, 1], F32, tag="tok", name="tk")
        nc.vector.memset(tk[:], 1.0)
        nc.sync.dma_start(tok_d[:], tk[:])
    ctxG.close()
    ctxZ.close()
    ctxL.close()


# ---------------- host glue ----------------

def _prep_inputs(inputs):
    x = np.asarray(inputs['x'], np.float32)
    xr = x.reshape(T, B, N, C)
    ident = np.eye(128, dtype=np.float16)
    mask = np.zeros((128, 512), np.float16)
    for blk in range(4):
        for h in range(4):
            mask[h * 32:(h + 1) * 32,
                 blk * 128 + h * 32:blk * 128 + (h + 1) * 32] = 0.125
    common = {"ident": ident, "mask": mask}
    for name in ("q", "k", "v", "p", "fc1", "fc2"):
        wdt = np.float16 if name in ("p", "fc2") else np.float32
        common[f"w_{name}"] = np.ascontiguousarray(
            np.asarray(inputs[name + "_w"]).T).astype(wdt)
    for name in ("q", "k", "v", "p", "fc1", "fc2"):
        common[f"pv_{name}"] = np.ascontiguousarray(np.stack(
            [np.asarray(inputs[name + "_g"], np.float32),
             np.asarray(inputs[name + "_be"], np.float32)], 1))
    common["pv_dw"] = np.ascontiguousarray(np.stack(
        [np.asarray(inputs["dw_g"], np.float32),
         np.asarray(inputs["dw_be"], np.float32)], 1))
    kv = np.asarray(inputs["dw_k"], np.float32).reshape(CH, 9)
    convd = np.zeros((54 * 128, 128), np.float16)
    for i in range(6):
        for k in range(9):
            blk = np.diag(kv[i * 128:(i + 1) * 128, k]).astype(np.float16)
            convd[(i * 9 + k) * 128:(i * 9 + k + 1) * 128, :] = blk
    common["convd"] = convd

    maps = []
    for c in range(NCORES):
        shard = xr[:, c * BS:(c + 1) * BS]
        xt = np.ascontiguousarray(shard.reshape(R, C).T)
        m = dict(common)
        m["xT"] = xt
        maps.append(m)
    return maps


_CACHE = {}


def _get_runner(debug_taps=False, timing=False, stop_after=None):
    key = (debug_taps, timing, stop_after)
    if key not in _CACHE:
        from runner_embed import SpmdRunner
        nc = build_kernel(debug_taps, timing, stop_after)
        _CACHE[key] = SpmdRunner(nc, NCORES)
    return _CACHE[key]


def kernel(**inputs):
    r = _get_runner()
    maps = _prep_inputs(inputs)
    args = r.prep(maps)
    outs = r.run(args)
    res = r.results(outs)
    full = np.empty((T, B, N, C), np.float32)
    for c in range(NCORES):
        o = res[c]["out"]
        full[:, c * BS:(c + 1) * BS] = o.T.reshape(T, BS, N, C)
    return np.ascontiguousarray(full.reshape(T * B, N, C))


# ---- embedded SPMD runner module ----
import types
runner_embed = types.ModuleType("runner_embed")
sys.modules["runner_embed"] = runner_embed
exec(r'''
import sys
sys.path.insert(0, '/opt/trn_rl_repo')
import numpy as np
import jax
from jax.sharding import Mesh, PartitionSpec, NamedSharding
from jax.experimental.shard_map import shard_map
import concourse.bass as bass
import concourse.mybir as mybir
from concourse.bass2jax import _bass_exec_p, install_neuronx_cc_hook, partition_id_tensor


class SpmdRunner:
    def __init__(self, nc, n_cores, repeat=1):
        install_neuronx_cc_hook()
        self.nc = nc
        self.n_cores = n_cores
        self.repeat = repeat
        partition_name = nc.partition_id_tensor.name if nc.partition_id_tensor else None
        in_names, out_names, out_avals, zero_outs = [], [], [], []
        for alloc in nc.m.functions[0].allocations:
            if not isinstance(alloc, mybir.MemoryLocationSet):
                continue
            name = alloc.memorylocations[0].name
            if alloc.kind == "ExternalInput":
                if name != partition_name:
                    in_names.append(name)
            elif alloc.kind == "ExternalOutput":
                shape = tuple(alloc.tensor_shape)
                dtype = mybir.dt.np(alloc.dtype)
                out_names.append(name)
                out_avals.append(jax.core.ShapedArray(shape, dtype))
                zero_outs.append(np.zeros(shape, dtype))
        self.in_names, self.out_names = in_names, out_names
        self.out_avals, self.zero_outs = out_avals, zero_outs
        n_params = len(in_names)
        n_outs = len(out_avals)
        all_in_names = list(in_names) + list(out_names)
        if partition_name is not None:
            all_in_names.append(partition_name)
        self.n_params = n_params

        nrep = self.repeat

        def _body(*args):
            operands = list(args)
            if partition_name is not None:
                operands.append(partition_id_tensor())
            all_outs = []
            for _ in range(nrep):
                outs = _bass_exec_p.bind(
                    *operands, out_avals=tuple(out_avals),
                    in_names=tuple(all_in_names), out_names=tuple(out_names),
                    lowering_input_output_aliases=(),
                    sim_require_finite=True, sim_require_nnan=True, nc=nc)
                all_outs.extend(outs)
                # chain: feed outputs back as the out-buffer operands of the
                # next call — defeats CSE and serializes the executions
                operands[n_params:n_params + n_outs] = list(outs)
            return tuple(all_outs)

        devices = jax.devices()[:n_cores]
        assert len(devices) == n_cores
        mesh = Mesh(np.asarray(devices), ("core",))
        self.mesh = mesh
        in_specs = (PartitionSpec("core"),) * (n_params + n_outs)
        out_specs = (PartitionSpec("core"),) * (n_outs * nrep)
        self.fn = jax.jit(
            shard_map(_body, mesh=mesh, in_specs=in_specs,
                      out_specs=out_specs, check_rep=False),
            keep_unused=True)

    def prep(self, in_maps):
        per_core = [[np.asarray(m[name]) for name in self.in_names]
                    for m in in_maps]
        concat_in = [np.concatenate([per_core[c][i] for c in range(self.n_cores)], axis=0)
                     for i in range(self.n_params)]
        concat_zeros = [np.zeros((self.n_cores * z.shape[0], *z.shape[1:]), z.dtype)
                        for z in self.zero_outs]
        sh = NamedSharding(self.mesh, PartitionSpec("core"))
        return [jax.device_put(a, sh) for a in concat_in + concat_zeros]

    def run(self, args):
        outs = self.fn(*args)
        jax.block_until_ready(outs)
        return outs

    def results(self, outs):
        res = []
        for c in range(self.n_cores):
            res.append({name: np.asarray(outs[i]).reshape(self.n_cores, *self.out_avals[i].shape)[c]
                        for i, name in enumerate(self.out_names)})
        return res

    def time_it(self, args, iters=20, warmup=3):
        import time
        for _ in range(warmup):
            self.run(args)
        ts = []
        for _ in range(iters):
            t0 = time.perf_counter()
            self.run(args)
            ts.append(time.perf_counter() - t0)
        ts = np.array(ts)
        return dict(min=ts.min(), median=float(np.median(ts)), mean=ts.mean())
''', runner_embed.__dict__)
